# revision 17
# baseline (speedup 1.0000x reference)
# kernel.py — Trainium2 Bass kernel for nn_DenseGridNet (bilinear grid sample + MLP)
#
# v3.1 strategy (data-parallel over 8 NeuronCores, sorted + PAIRED points):
#  * host: computes cell ids + bilinear coefs (exact fp32 replication of the
#    reference), sorts points by cell, PAIRS adjacent points whose cells are
#    <=1 apart (~3% dummy padding), and uploads:
#      - ptw: DENSELY PREPACKED per-pair 64B patch blocks (the pair's two
#        16-value cell patches), laid out [chunk][partition][pair-in-row].
#        This replaces v2's windowed dma_gather (256B descriptors, 2x small-
#        transfer penalty, SWDGE descriptor generation on GpSimd) with a
#        plain contiguous 2KB-per-partition DMA: ~8x less DMA-engine time.
#      - fp16 c8 coefs (4 bilinear coefs placed in the pair-half the point
#        occupies, zeros in the other half),
#      - fp16 idf values in a partition-major layout.
#  * device per 4096-pair chunk:
#      - one contiguous DMA loads the 64B patch blocks -> pb [128,32,32];
#      - GpSimd fp16 multiply q = c8 (x) patch, written STRIDED into a
#        128-slot-per-pair q tile: slots 0:64 = q products, 64:66 = the
#        pair's idf values (tiny DVE copy), 66 = constant 1.0 (memset once),
#        67:128 = zeros (memset once);
#      - one xbar DMA transpose of the q tile: because the pair unit is 128
#        slots, pair-slot s lands exactly at partition s, so the idf/ones
#        rows ride THROUGH the transpose and layer1 needs a SINGLE K=67
#        matmul per 512 columns (w1x slots + idf*w1[0] + b1 fold): half of
#        v2's layer-1 PE passes, and no extra K=2 idf matmul;
#      - layer2 block-diagonal (2 points/col, K=128);
#      - layer3 packs 8 points/col: 4 accumulating stride-4 matmuls into a
#        [24,256] psum tile, so sigmoid free-size is 4x smaller than v2;
#      - relu work split across DVE/ACT, qmult on GpSimd (it cannot read
#        PSUM so it gets no relu work), sigmoid on ACT, fp16 output.
import os
import numpy as np

RX = 1024
RY = 1024
F = 4
HID = 64
N_CORES = 8
P = 128          # partitions
SLOT = 32        # fp16 q slots per point ([h][f][j] of the 2-cell pair)
PU = 128         # padded pair unit in the q tile (64 q + idfA + idfB + 1 + pad)
CHPAIRS = 4096   # pairs per chunk
CHPTS = 8192     # points per chunk (2 per pair, incl. dummies)
GCH = CHPAIRS // P            # 32 pair rows per partition per chunk
TCH = CHPTS // P              # 64 point cols per chunk


def _build_bass(n_chunks, front_swap=False):
    """Bass program for one core processing n_chunks*CHPTS (padded) points."""
    import concourse.bass as bass
    import concourse.tile as tile
    from concourse import bacc
    import concourse.mybir as mybir

    dt = mybir.dt

    nc = bacc.Bacc(None, target_bir_lowering=False)

    f32 = dt.float32
    f16 = dt.float16

    # ---- DRAM I/O -------------------------------------------------------
    ptw_d = nc.dram_tensor("ptw", [n_chunks * P, GCH * SLOT], f16,
                           kind="ExternalInput")
    co8_d = nc.dram_tensor("co8", [P, n_chunks * TCH, 8], f16,
                           kind="ExternalInput")
    idfc_d = nc.dram_tensor("idfc", [P, n_chunks * GCH * 2], f16,
                            kind="ExternalInput")
    l1_d = nc.dram_tensor("lhsT1", [67, 128], f16, kind="ExternalInput")
    l2_d = nc.dram_tensor("lhsT2", [128, 128], f16, kind="ExternalInput")
    l3_d = nc.dram_tensor("lhsT3", [128, 96], f16, kind="ExternalInput")
    b2_d = nc.dram_tensor("b2rep", [128, 1], f32, kind="ExternalInput")
    b3_d = nc.dram_tensor("b3rep", [24, 1], f32, kind="ExternalInput")
    out_d = nc.dram_tensor("yT", [24, n_chunks * 1024], f16,
                           kind="ExternalOutput")

    with tile.TileContext(nc) as tc:
        with (
            tc.tile_pool(name="persist", bufs=1) as pp,
            tc.tile_pool(name="psum_l1", bufs=2, space="PSUM") as ps1,
            tc.tile_pool(name="psum_l2", bufs=2, space="PSUM") as ps2,
            tc.tile_pool(name="psum_l3", bufs=2, space="PSUM") as ps3,
            tc.tile_pool(name="mlp", bufs=3) as mp,
            tc.tile_pool(name="mlp2", bufs=3) as mp2,
            tc.tile_pool(name="outp", bufs=3) as op_,
        ):
            l1w = pp.tile([67, 128], f16, tag="l1w")
            l2w = pp.tile([128, 128], f16, tag="l2w")
            l3w = pp.tile([128, 96], f16, tag="l3w")
            b2r = pp.tile([128, 1], f32, tag="b2r")
            b3r = pp.tile([24, 1], f32, tag="b3r")

            NPB = 8
            NQ = 4
            NTB = 4
            pb = [pp.tile([P, GCH, SLOT], f16, name=f"pb{i}", tag=f"pb{i}")
                  for i in range(NPB)]
            qpm = [pp.tile([P, GCH, PU], f16, name=f"qpm{i}", tag=f"qpm{i}")
                   for i in range(NQ)]
            tb = [pp.tile([P, GCH, P], f16, name=f"tb{i}", tag=f"tb{i}")
                  for i in range(NTB)]
            co8s = pp.tile([P, n_chunks * TCH, 8], f16, tag="co8s")
            idfc = pp.tile([P, n_chunks * GCH * 2], f16, tag="idfc")

            nc.sync.dma_start(l1w[:], l1_d[:])
            nc.sync.dma_start(l2w[:], l2_d[:])
            nc.sync.dma_start(l3w[:], l3_d[:])
            nc.sync.dma_start(b2r[:], b2_d[:])
            nc.sync.dma_start(b3r[:], b3_d[:])
            nc.sync.dma_start(co8s[:], co8_d[:])
            nc.sync.dma_start(idfc[:], idfc_d[:])
            for i in range(NQ):
                # pair-unit slots 66 (=1.0 for the b1 fold) and 67:128
                # (zeros) are constant across chunks: written once.
                nc.vector.memset(qpm[i][:, :, 66:67], 1.0)
                nc.vector.memset(qpm[i][:, :, 67:PU], 0.0)

            AL = mybir.AluOpType
            ACTF = mybir.ActivationFunctionType

            def issue_pb(chi):
                """patch-pair block load for chunk chi. Issued from the DVE
                hwdge queue: SP's in-order queue holds the transposes, which
                wait on qmult sems — pb loads must not sit behind them."""
                nc.scalar.dma_start(
                    pb[chi % NPB][:],
                    ptw_d[chi * P:(chi + 1) * P, :]
                    .rearrange("p (g s) -> p g s", s=SLOT))

            def issue_qmult(chi):
                """q = c8 (x) patch for chunk chi, plus the idf slot copy."""
                pbb = pb[chi % NPB]
                qb = qpm[chi % NQ]
                c0 = chi * TCH
                in1 = (co8s[:, c0:c0 + TCH, :]
                       .rearrange("p (g d) (h j) -> p g d h j", d=2, h=2)
                       .unsqueeze(4)
                       .to_broadcast([P, GCH, 2, 2, 4, 4]))
                in2 = (pbb[:]
                       .rearrange("p g (h f j) -> p g h f j", h=2, f=4)
                       .unsqueeze(2)
                       .to_broadcast([P, GCH, 2, 2, 4, 4]))
                outq = (qb[:, :, 0:64]
                        .rearrange("p g (d h f j) -> p g d h f j",
                                   d=2, h=2, f=4))
                # GpSimd: the only engine with no psum-side relu work
                # (gpsimd cannot read PSUM), so q-mult goes here.
                nc.gpsimd.tensor_tensor(outq, in1, in2, AL.mult)
                # pair idf values -> slots 64:66 (ride through the transpose)
                nc.vector.tensor_copy(
                    qb[:, :, 64:66],
                    idfc[:, chi * GCH * 2:(chi + 1) * GCH * 2]
                    .rearrange("p (g d) -> p g d", d=2))

            def issue_transpose(chi):
                nc.sync.dma_start_transpose(
                    tb[chi % NTB][:],
                    qpm[chi % NQ][:].rearrange("p g s -> p (g s)"))

            # two-deep front pipeline: the q chain (qmult ~4.2us + transpose
            # ~3.6us) exceeds one chunk period, so qmult runs two chunks
            # ahead of the MLP and the transpose one chunk ahead.
            for i in range(5):
                if i < n_chunks:
                    issue_pb(i)
            issue_qmult(0)
            issue_transpose(0)
            issue_qmult(1)
            for chi in range(n_chunks):
                tbb = tb[chi % NTB]
                if chi + 2 < n_chunks:
                    issue_qmult(chi + 2)
                if chi + 1 < n_chunks:
                    issue_transpose(chi + 1)
                if chi + 5 < n_chunks:
                    issue_pb(chi + 5)

                y3c = op_.tile([24, 1024], f16, tag="y3c")
                H = 512
                l1pp = {}
                h1t = {}
                h2t = {}
                # skewed emission: each stage's input is >=2 steps old,
                # hiding relu/psum handoff latency.
                for sq in range(14):
                    if sq < 8:
                        qq, half = sq // 2, sq % 2
                        if half == 0:
                            l1pp[qq] = ps1.tile([P, 1024], f32, name="l1p",
                                                tag="l1p")
                        l1p = l1pp[qq]
                        # single K=67 pass: q slots + idf rows + ones(b1)
                        nc.tensor.matmul(l1p[:, half * H:half * H + H],
                                         l1w[:],
                                         tbb[0:67, 4 * sq:4 * sq + 4, :],
                                         start=True, stop=True)
                    if 2 <= sq < 10 and (sq - 2) % 2 == 0:
                        qq = (sq - 2) // 2
                        l1p = l1pp.pop(qq)
                        h1 = mp.tile([P, 1024], f16, tag="h1")
                        # relu1: b1 already folded into the matmul ones-row
                        if qq == 0:
                            nc.vector.tensor_scalar(
                                h1[:], l1p[:], 0.0, None, AL.max)
                        else:
                            nc.scalar.activation(h1[:], l1p[:], ACTF.Relu)
                        h1t[qq] = h1
                    if 3 <= sq < 11:
                        s1 = sq - 3
                        qq, hh = s1 // 2, s1 % 2
                        h1 = h1t[qq]
                        l2p = ps2.tile([P, H], f32, tag="l2p")
                        nc.tensor.matmul(l2p[:], l2w[:],
                                         h1[:, hh * H:hh * H + H],
                                         start=True, stop=True)
                        if hh == 1:
                            h1t.pop(qq)
                        if hh == 0:
                            h2t[qq] = mp2.tile([P, 1024], f16, name="h2",
                                               tag="h2")
                        h2 = h2t[qq]
                        if s1 in (1, 5):
                            nc.scalar.activation(
                                h2[:, hh * H:hh * H + H], l2p[:],
                                ACTF.Relu, bias=b2r[:])
                        else:
                            nc.vector.tensor_scalar(
                                h2[:, hh * H:hh * H + H], l2p[:],
                                b2r[:], 0.0, AL.add, AL.max)
                    if sq >= 6 and (sq - 6) % 2 == 0:
                        qq = (sq - 6) // 2
                        r, rh = qq // 2, qq % 2
                        h2 = h2t.pop(qq)
                        if rh == 0:
                            l3pp = ps3.tile([24, 512], f32, name="l3p",
                                            tag="l3p")
                        h2v = h2[:].rearrange("p (u e) -> p e u", e=4)
                        # one 8-matmul accumulation group per [24,512] bank:
                        # quarters 2r (cols 0:256) and 2r+1 (cols 256:512)
                        for e in range(4):
                            nc.tensor.matmul(
                                l3pp[:, rh * 256:rh * 256 + 256],
                                l3w[:, 24 * e:24 * e + 24],
                                h2v[:, e, :],
                                start=(rh == 0 and e == 0),
                                stop=(rh == 1 and e == 3),
                                skip_group_check=True)
                        if rh == 1:
                            nc.scalar.activation(
                                y3c[:, 512 * r:512 * r + 512],
                                l3pp[:], ACTF.Sigmoid, bias=b3r[:])
                nc.scalar.dma_start(
                    out_d[:, chi * 1024:(chi + 1) * 1024], y3c[:])

    return nc


# ======================= host-side preparation ==========================

def _host_pointdata(x):
    """Exact fp32 replication of the reference's per-point index/weight math.

    Returns (cell[int64], c4[N,4] fp32 in corner order 00,10,01,11, idf)."""
    u = np.asarray(x[:, 1], np.float32)
    v = np.asarray(x[:, 2], np.float32)
    xu = u * np.float32(RX)
    yv = v * np.float32(RY)
    x0 = xu.astype(np.int32)
    x0 = np.where(x0 == RX, 0, x0)
    y0 = yv.astype(np.int32)
    wx = xu - x0.astype(np.float32)
    wy = yv - y0.astype(np.float32)
    cell = np.minimum(y0.astype(np.int64) * RX + x0, RX * RY - 1)
    one = np.float32(1.0)
    c00 = (one - wx) * (one - wy)
    c10 = wx * (one - wy)
    c01 = (one - wx) * wy
    c11 = wx * wy
    c4 = np.stack([c00, c10, c01, c11], axis=1)
    return cell, c4, np.asarray(x[:, 0], np.float32)


def _patch_table16(emb):
    """[RX*RY + 8, 16] fp16: per-cell patch in [f][j] order (j: 00,10,01,11),
    with x/y edge clamping baked in; 8 zero pad rows."""
    e = np.asarray(emb, dtype=np.float32).reshape(RY, RX, F)
    xs = np.arange(RX)
    x1 = np.minimum(xs + 1, RX - 1)
    ys = np.arange(RY)
    y1 = np.minimum(ys + 1, RY - 1)
    p = np.empty((RY, RX, F, 4), dtype=np.float32)   # [y, x, f, j]
    p[:, :, :, 0] = e
    p[:, :, :, 1] = e[:, x1, :]
    p[:, :, :, 2] = e[y1, :, :]
    p[:, :, :, 3] = e[y1][:, x1, :]
    pt = np.zeros((RY * RX + 8, 16), dtype=np.float16)
    pt[:RY * RX] = p.reshape(RY * RX, 16).astype(np.float16)
    return pt


def _pairing(cell_sorted):
    """Pair consecutive-cell points. Returns (pA, pB, base) index arrays into
    the SORTED order; pB == -1 for half-dummy pairs."""
    N = len(cell_sorted)
    counts = np.bincount(cell_sorted, minlength=RX * RY)
    starts = np.zeros(RX * RY + 1, np.int64)
    np.cumsum(counts, out=starts[1:])
    wi = np.arange(N) - starts[cell_sorted]          # within-cell rank
    cnt = counts[cell_sorted]
    inA = (wi % 2 == 0) & (wi + 1 < cnt)             # within-cell pair firsts
    pA_in = np.nonzero(inA)[0]
    pB_in = pA_in + 1
    # leftover points (one per odd-count cell), ordered by cell
    lmask = (wi == cnt - 1) & (cnt % 2 == 1)
    lpos = np.nonzero(lmask)[0]
    lcell = cell_sorted[lpos]
    # pair leftovers within runs of consecutive cells
    if len(lpos):
        newrun = np.r_[True, np.diff(lcell) != 1]
        runid = np.cumsum(newrun) - 1
        rstart = np.nonzero(newrun)[0]
        within = np.arange(len(lpos)) - rstart[runid]
        runlen = np.bincount(runid)
        isA = (within % 2 == 0) & (within + 1 < runlen[runid])
        pA_lo = lpos[isA]
        pB_lo = lpos[np.nonzero(isA)[0] + 1]
        solo = (within == runlen[runid] - 1) & (runlen[runid] % 2 == 1)
        pA_solo = lpos[solo]
    else:
        pA_lo = pB_lo = pA_solo = np.zeros(0, np.int64)
    pA = np.concatenate([pA_in, pA_lo, pA_solo])
    pB = np.concatenate([pB_in, pB_lo, np.full(len(pA_solo), -1, np.int64)])
    base = cell_sorted[pA]
    o = np.argsort(base, kind="stable")
    return pA[o], pB[o], base[o]


def _host_prep_weights(w1, b1, w2, b2, w3, b3):
    w1 = np.asarray(w1, np.float32)
    # w1x rows s = h*16 + f*4 + j -> w1[1+f]
    w1x = np.zeros((SLOT, HID), np.float32)
    for h in range(2):
        for f in range(4):
            for j in range(4):
                w1x[h * 16 + f * 4 + j] = w1[1 + f]
    lhsT1 = np.zeros((67, 128), np.float16)
    lhsT1[0:32, 0:64] = w1x          # even point slots
    lhsT1[32:64, 64:128] = w1x       # odd point slots
    lhsT1[64, 0:64] = w1[0]          # even idf
    lhsT1[65, 64:128] = w1[0]        # odd idf
    lhsT1[66, 0:64] = np.asarray(b1, np.float32)   # ones row -> +b1
    lhsT1[66, 64:128] = np.asarray(b1, np.float32)
    lhsT2 = np.zeros((128, 128), np.float16)
    lhsT2[0:64, 0:64] = w2
    lhsT2[64:128, 64:128] = w2
    lhsT3 = np.zeros((128, 96), np.float16)
    for e in range(4):
        lhsT3[0:64, 24 * e + 6 * e:24 * e + 6 * e + 3] = w3
        lhsT3[64:128, 24 * e + 6 * e + 3:24 * e + 6 * e + 6] = w3
    b2rep = np.concatenate([b2, b2]).astype(np.float32).reshape(128, 1)
    b3rep = np.zeros((24, 1), np.float32)
    for e in range(4):
        b3rep[6 * e:6 * e + 3, 0] = b3
        b3rep[6 * e + 3:6 * e + 6, 0] = b3
    return lhsT1, lhsT2, lhsT3, b2rep, b3rep


def _colmaps():
    """Column maps for the v3 psum/output layout.

    Returns jc_e: [4, 1024] int64 — for output column v in [0,1024) and
    row-block e in [0,4): the within-chunk PAIR index jc whose points occupy
    y3c rows 6e..6e+3 (A) and 6e+3..6e+6 (B) of column v."""
    v = np.arange(1024)
    t = v // 256          # h2 tile (= psum quarter) index
    uu = v % 256          # octet index within the tile
    jc_e = np.empty((4, 1024), np.int64)
    for e in range(4):
        Cc = 1024 * t + 4 * uu + e          # chunk psum column
        qq = Cc // 1024
        rr = Cc % 1024
        half = rr // 512
        cc = (rr % 512) // 128
        p = rr % 128
        g = 8 * qq + 4 * half + cc
        jc_e[e] = g * 128 + p
    return jc_e


def _prep_in_maps(x, emb, w1, b1, w2, b2, w3, b3):
    x = np.asarray(x, np.float32)
    cell, c4, idf = _host_pointdata(x)
    order = np.argsort(cell, kind="stable")
    cs = cell[order]
    pA_s, pB_s, base = _pairing(cs)          # indices into sorted order
    npair_real = len(pA_s)

    n_chunks = -(-npair_real // (N_CORES * CHPAIRS))
    npairs = n_chunks * N_CORES * CHPAIRS
    PAIRS_CORE = n_chunks * CHPAIRS

    # orig-index pair arrays, padded with full dummies
    pA = np.full(npairs, -1, np.int64)
    pB = np.full(npairs, -1, np.int64)
    bases = np.full(npairs, int(base[-1]) if npair_real else 0, np.int64)
    pA[:npair_real] = order[pA_s]
    pB[:npair_real] = np.where(pB_s >= 0, order[np.maximum(pB_s, 0)], -1)
    bases[:npair_real] = base

    # per-point halves: h for B points (0 or 1); A is always h=0
    validB = pB >= 0
    hB = np.zeros(npairs, np.int64)
    hB[validB] = cell[pB[validB]] - bases[validB]
    assert hB.min() >= 0 and hB.max() <= 1

    # c8 per pair half [npairs, 2, 8] fp16
    c8 = np.zeros((npairs, 2, 8), np.float16)
    vA = pA >= 0
    c8[vA, 0, 0:4] = c4[pA[vA]].astype(np.float16)
    c8[validB, 1, 0:4] = np.where((hB[validB] == 0)[:, None],
                                  c4[pB[validB]], 0).astype(np.float16)
    c8[validB, 1, 4:8] = np.where((hB[validB] == 1)[:, None],
                                  c4[pB[validB]], 0).astype(np.float16)
    idfA = np.where(vA, idf[np.maximum(pA, 0)], 0).astype(np.float16)
    idfB = np.where(validB, idf[np.maximum(pB, 0)], 0).astype(np.float16)

    pt16 = _patch_table16(emb)
    # densely prepacked per-pair patch blocks: [npairs, 32] fp16
    pp32 = np.empty((npairs, 2 * 16), np.float16)
    pp32[:, 0:16] = pt16[bases]
    pp32[:, 16:32] = pt16[bases + 1]

    lhsT1, lhsT2, lhsT3, b2rep, b3rep = _host_prep_weights(
        w1, b1, w2, b2, w3, b3)
    jc_e = _colmaps()

    in_maps = []
    out_pairs = []                            # (origA[4,ncol], origB) per core
    for k in range(N_CORES):
        s = k * PAIRS_CORE
        # ptw: [n_chunks*P, GCH*SLOT]: row (c, p), block g = pair jc=g*128+p
        kpp = pp32[s:s + PAIRS_CORE].reshape(n_chunks, GCH, P, SLOT)
        ptw = np.ascontiguousarray(
            kpp.transpose(0, 2, 1, 3).reshape(n_chunks * P, GCH * SLOT))

        # co8: [P, n_chunks*64, 8] : pair jc=(g*128+p) -> co8[p, chunk*64+2g+d]
        kc8 = c8[s:s + PAIRS_CORE].reshape(n_chunks, GCH, P, 2, 8)
        co8 = np.ascontiguousarray(
            kc8.transpose(2, 0, 1, 3, 4).reshape(P, n_chunks * TCH, 8))

        # idfc: [P, n_chunks*GCH*2]: idfc[p, (c, g, d)] = idf of point d of
        # pair jc = c*4096 + g*128 + p
        kiA = idfA[s:s + PAIRS_CORE].reshape(n_chunks, GCH, P)
        kiB = idfB[s:s + PAIRS_CORE].reshape(n_chunks, GCH, P)
        kid = np.stack([kiA, kiB], axis=3)        # [c, g, p, d] -> wait
        # kid[c, g, p, d]; want idfc[p, c, g, d]
        idfc = np.ascontiguousarray(
            kid.transpose(2, 0, 1, 3).reshape(P, n_chunks * GCH * 2))

        # output column -> orig point indices: [4, n_chunks*1024] each
        kpA = pA[s:s + PAIRS_CORE].reshape(n_chunks, CHPAIRS)
        kpB = pB[s:s + PAIRS_CORE].reshape(n_chunks, CHPAIRS)
        oA = np.empty((4, n_chunks * 1024), np.int64)
        oB = np.empty((4, n_chunks * 1024), np.int64)
        for e in range(4):
            oA[e] = kpA[:, jc_e[e]].reshape(-1)
            oB[e] = kpB[:, jc_e[e]].reshape(-1)
        out_pairs.append((oA, oB))

        in_maps.append({
            "ptw": ptw,
            "co8": co8,
            "idfc": idfc,
            "lhsT1": lhsT1,
            "lhsT2": lhsT2,
            "lhsT3": lhsT3,
            "b2rep": b2rep,
            "b3rep": b3rep,
        })
    return in_maps, out_pairs, n_chunks


_CACHE = {}


def kernel(x, emb, w1, b1, w2, b2, w3, b3):
    from concourse.bass_utils import run_bass_kernel_spmd

    x = np.asarray(x, np.float32)
    N = x.shape[0]

    in_maps, out_pairs, n_chunks = _prep_in_maps(x, emb, w1, b1, w2, b2, w3, b3)

    key = (n_chunks,)
    if key not in _CACHE:
        nc_new = _build_bass(n_chunks)
        nc_new.compile()
        _CACHE[key] = nc_new
    nc = _CACHE[key]

    trace = os.environ.get("KERNEL_TRACE", "0") == "1"
    res = run_bass_kernel_spmd(
        nc, in_maps, core_ids=list(range(N_CORES)), trace=trace
    )
    if trace and res.exec_time_ns is not None:
        print(f"HW exec time: {res.exec_time_ns} ns")

    y = np.empty((N, 3), np.float32)
    for k in range(N_CORES):
        yT = np.asarray(res.results[k]["yT"], np.float32)
        oA, oB = out_pairs[k]
        for e in range(4):
            mA = oA[e] >= 0
            mB = oB[e] >= 0
            y[oA[e][mA], :] = yT[6 * e:6 * e + 3, mA].T
            y[oB[e][mB], :] = yT[6 * e + 3:6 * e + 6, mB].T
    return y


# revision 19
# speedup vs baseline: 1.0826x; 1.0826x over previous
# kernel.py — Trainium2 Bass kernel for nn_DenseGridNet (bilinear grid sample + MLP)
#
# v3.1 strategy (data-parallel over 8 NeuronCores, sorted + PAIRED points):
#  * host: computes cell ids + bilinear coefs (exact fp32 replication of the
#    reference), sorts points by cell, PAIRS adjacent points whose cells are
#    <=1 apart (~3% dummy padding), and uploads:
#      - ptw: DENSELY PREPACKED per-pair 64B patch blocks (the pair's two
#        16-value cell patches), laid out [chunk][partition][pair-in-row].
#        This replaces v2's windowed dma_gather (256B descriptors, 2x small-
#        transfer penalty, SWDGE descriptor generation on GpSimd) with a
#        plain contiguous 2KB-per-partition DMA: ~8x less DMA-engine time.
#      - fp16 c8 coefs (4 bilinear coefs placed in the pair-half the point
#        occupies, zeros in the other half),
#      - fp16 idf values in a partition-major layout.
#  * device per 4096-pair chunk:
#      - one contiguous DMA loads the 64B patch blocks -> pb [128,32,32];
#      - GpSimd fp16 multiply q = c8 (x) patch, written STRIDED into a
#        128-slot-per-pair q tile: slots 0:64 = q products, 64:66 = the
#        pair's idf values (tiny DVE copy), 66 = constant 1.0 (memset once),
#        67:128 = zeros (memset once);
#      - one xbar DMA transpose of the q tile: because the pair unit is 128
#        slots, pair-slot s lands exactly at partition s, so the idf/ones
#        rows ride THROUGH the transpose and layer1 needs a SINGLE K=67
#        matmul per 512 columns (w1x slots + idf*w1[0] + b1 fold): half of
#        v2's layer-1 PE passes, and no extra K=2 idf matmul;
#      - layer2 block-diagonal (2 points/col, K=128);
#      - layer3 packs 8 points/col: 4 accumulating stride-4 matmuls into a
#        [24,256] psum tile, so sigmoid free-size is 4x smaller than v2;
#      - relu work split across DVE/ACT, qmult on GpSimd (it cannot read
#        PSUM so it gets no relu work), sigmoid on ACT, fp16 output.
import os
import numpy as np

RX = 1024
RY = 1024
F = 4
HID = 64
N_CORES = 8
P = 128          # partitions
SLOT = 32        # fp16 q slots per point ([h][f][j] of the 2-cell pair)
PU = 128         # padded pair unit in the q tile (64 q + idfA + idfB + 1 + pad)
CHPAIRS = 4096   # pairs per chunk
CHPTS = 8192     # points per chunk (2 per pair, incl. dummies)
GCH = CHPAIRS // P            # 32 pair rows per partition per chunk
TCH = CHPTS // P              # 64 point cols per chunk


def _build_bass(n_chunks, front_swap=False):
    """Bass program for one core processing n_chunks*CHPTS (padded) points."""
    import concourse.bass as bass
    import concourse.tile as tile
    from concourse import bacc
    import concourse.mybir as mybir

    dt = mybir.dt

    nc = bacc.Bacc(None, target_bir_lowering=False)

    f32 = dt.float32
    f16 = dt.float16

    # ---- DRAM I/O -------------------------------------------------------
    ptw_d = nc.dram_tensor("ptw", [n_chunks * P, GCH * SLOT], f16,
                           kind="ExternalInput")
    co8_d = nc.dram_tensor("co8", [P, n_chunks * TCH, 8], f16,
                           kind="ExternalInput")
    idfc_d = nc.dram_tensor("idfc", [P, n_chunks * GCH * 2], f16,
                            kind="ExternalInput")
    l1_d = nc.dram_tensor("lhsT1", [67, 128], f16, kind="ExternalInput")
    l2_d = nc.dram_tensor("lhsT2", [128, 128], f16, kind="ExternalInput")
    l3_d = nc.dram_tensor("lhsT3", [128, 96], f16, kind="ExternalInput")
    b2_d = nc.dram_tensor("b2rep", [128, 1], f32, kind="ExternalInput")
    b3_d = nc.dram_tensor("b3rep", [24, 1], f32, kind="ExternalInput")
    out_d = nc.dram_tensor("yT", [24, n_chunks * 1024], f16,
                           kind="ExternalOutput")

    with tile.TileContext(nc) as tc:
        with (
            tc.tile_pool(name="persist", bufs=1) as pp,
            tc.tile_pool(name="psum_l1", bufs=2, space="PSUM") as ps1,
            tc.tile_pool(name="psum_l2", bufs=2, space="PSUM") as ps2,
            tc.tile_pool(name="psum_l3", bufs=2, space="PSUM") as ps3,
            tc.tile_pool(name="mlp", bufs=3) as mp,
            tc.tile_pool(name="mlp2", bufs=4) as mp2,
            tc.tile_pool(name="outp", bufs=3) as op_,
        ):
            l1w = pp.tile([67, 128], f16, tag="l1w")
            l2w = pp.tile([128, 128], f16, tag="l2w")
            l3w = pp.tile([128, 96], f16, tag="l3w")
            b2r = pp.tile([128, 1], f32, tag="b2r")
            b3r = pp.tile([24, 1], f32, tag="b3r")

            NPB = 8
            NQ = 4
            NTB = 4
            pb = [pp.tile([P, GCH, SLOT], f16, name=f"pb{i}", tag=f"pb{i}")
                  for i in range(NPB)]
            qpm = [pp.tile([P, GCH, PU], f16, name=f"qpm{i}", tag=f"qpm{i}")
                   for i in range(NQ)]
            tb = [pp.tile([P, GCH, P], f16, name=f"tb{i}", tag=f"tb{i}")
                  for i in range(NTB)]
            co8s = pp.tile([P, n_chunks * TCH, 8], f16, tag="co8s")
            idfc = pp.tile([P, n_chunks * GCH * 2], f16, tag="idfc")

            nc.sync.dma_start(l1w[:], l1_d[:])
            nc.sync.dma_start(l2w[:], l2_d[:])
            nc.sync.dma_start(l3w[:], l3_d[:])
            nc.sync.dma_start(b2r[:], b2_d[:])
            nc.sync.dma_start(b3r[:], b3_d[:])
            nc.sync.dma_start(co8s[:], co8_d[:])
            nc.sync.dma_start(idfc[:], idfc_d[:])
            for i in range(NQ):
                # pair-unit slots 66 (=1.0 for the b1 fold) and 67:128
                # (zeros) are constant across chunks: written once.
                nc.vector.memset(qpm[i][:, :, 66:67], 1.0)
                nc.vector.memset(qpm[i][:, :, 67:PU], 0.0)

            AL = mybir.AluOpType
            ACTF = mybir.ActivationFunctionType

            def issue_pb(chi):
                """patch-pair block load for chunk chi. Issued from the DVE
                hwdge queue: SP's in-order queue holds the transposes, which
                wait on qmult sems — pb loads must not sit behind them."""
                nc.scalar.dma_start(
                    pb[chi % NPB][:],
                    ptw_d[chi * P:(chi + 1) * P, :]
                    .rearrange("p (g s) -> p g s", s=SLOT))

            def issue_qmult(chi):
                """q = c8 (x) patch for chunk chi, plus the idf slot copy."""
                pbb = pb[chi % NPB]
                qb = qpm[chi % NQ]
                c0 = chi * TCH
                in1 = (co8s[:, c0:c0 + TCH, :]
                       .rearrange("p (g d) (h j) -> p g d h j", d=2, h=2)
                       .unsqueeze(4)
                       .to_broadcast([P, GCH, 2, 2, 4, 4]))
                in2 = (pbb[:]
                       .rearrange("p g (h f j) -> p g h f j", h=2, f=4)
                       .unsqueeze(2)
                       .to_broadcast([P, GCH, 2, 2, 4, 4]))
                outq = (qb[:, :, 0:64]
                        .rearrange("p g (d h f j) -> p g d h f j",
                                   d=2, h=2, f=4))
                # GpSimd: the only engine with no psum-side relu work
                # (gpsimd cannot read PSUM), so q-mult goes here.
                nc.gpsimd.tensor_tensor(outq, in1, in2, AL.mult)
                # pair idf values -> slots 64:66 (ride through the transpose)
                nc.vector.tensor_copy(
                    qb[:, :, 64:66],
                    idfc[:, chi * GCH * 2:(chi + 1) * GCH * 2]
                    .rearrange("p (g d) -> p g d", d=2))

            def issue_transpose(chi):
                nc.sync.dma_start_transpose(
                    tb[chi % NTB][:],
                    qpm[chi % NQ][:].rearrange("p g s -> p (g s)"))

            # two-deep front pipeline: the q chain (qmult ~4.2us + transpose
            # ~3.6us) exceeds one chunk period, so qmult runs two chunks
            # ahead of the MLP and the transpose one chunk ahead.
            # ---- flat software pipeline over global quarter-steps --------
            # Engines execute their queues IN ORDER, so chunk-sequential
            # emission makes every chunk's slow tail (L3->sigmoid) block the
            # next chunk's head. Instead, one step s emits: L1 of quarter s,
            # relu1 of s-1, L2+relu2 of s-2, L3(+sigmoid) of s-4 — each
            # engine's queue then interleaves 3-4 chunks and every
            # dependency is satisfied about a full step before its consumer.
            H = 512
            NQTR = 4 * n_chunks
            l1pp = {}
            h1t = {}
            h2t = {}
            l3pt = {}
            y3t = {}

            def front(k):
                if k + 2 < n_chunks:
                    issue_qmult(k + 2)
                if k + 1 < n_chunks:
                    issue_transpose(k + 1)
                if k + 5 < n_chunks:
                    issue_pb(k + 5)

            def st_l1(Q):
                k, qq = Q // 4, Q % 4
                tbb = tb[k % NTB]
                l1p = ps1.tile([P, 1024], f32, name="l1p", tag="l1p")
                l1pp[Q] = l1p
                for half in range(2):
                    m = 2 * qq + half
                    # single K=67 pass: q slots + idf rows + ones(b1)
                    nc.tensor.matmul(l1p[:, half * H:half * H + H],
                                     l1w[:],
                                     tbb[0:67, 4 * m:4 * m + 4, :],
                                     start=True, stop=True)

            def st_relu1(Q):
                l1p = l1pp.pop(Q)
                h1 = mp.tile([P, 1024], f16, tag="h1")
                # relu1: b1 already folded into the matmul ones-row
                if Q % 4 == 0:
                    nc.vector.tensor_scalar(h1[:], l1p[:], 0.0, None, AL.max)
                else:
                    nc.scalar.activation(h1[:], l1p[:], ACTF.Relu)
                h1t[Q] = h1

            def st_l2(Q):
                h1 = h1t.pop(Q)
                h2 = mp2.tile([P, 1024], f16, name="h2", tag="h2")
                h2t[Q] = h2
                for hh in range(2):
                    l2p = ps2.tile([P, H], f32, tag="l2p")
                    nc.tensor.matmul(l2p[:], l2w[:],
                                     h1[:, hh * H:hh * H + H],
                                     start=True, stop=True)
                    if Q % 2 == 1 and hh == 1:
                        nc.scalar.activation(
                            h2[:, hh * H:hh * H + H], l2p[:],
                            ACTF.Relu, bias=b2r[:])
                    else:
                        nc.vector.tensor_scalar(
                            h2[:, hh * H:hh * H + H], l2p[:],
                            b2r[:], 0.0, AL.add, AL.max)

            def st_l3(Q):
                k, qq = Q // 4, Q % 4
                r, rh = qq // 2, qq % 2
                h2 = h2t.pop(Q)
                if rh == 0:
                    l3pt[Q // 2] = ps3.tile([24, 512], f32, name="l3p",
                                            tag="l3p")
                if qq == 0:
                    y3t[k] = op_.tile([24, 1024], f16, name="y3c", tag="y3c")
                l3pp = l3pt[Q // 2]
                h2v = h2[:].rearrange("p (u e) -> p e u", e=4)
                # one 8-matmul accumulation group per [24,512] bank:
                # quarters 2r (cols 0:256) and 2r+1 (cols 256:512)
                for e in range(4):
                    nc.tensor.matmul(
                        l3pp[:, rh * 256:rh * 256 + 256],
                        l3w[:, 24 * e:24 * e + 24],
                        h2v[:, e, :],
                        start=(rh == 0 and e == 0),
                        stop=(rh == 1 and e == 3),
                        skip_group_check=True)
                if rh == 1:
                    l3pp = l3pt.pop(Q // 2)
                    nc.scalar.activation(
                        y3t[k][:, 512 * r:512 * r + 512],
                        l3pp[:], ACTF.Sigmoid, bias=b3r[:])
                if qq == 3:
                    y3c = y3t.pop(k)
                    nc.scalar.dma_start(
                        out_d[:, k * 1024:(k + 1) * 1024], y3c[:])

            for i in range(5):
                if i < n_chunks:
                    issue_pb(i)
            issue_qmult(0)
            issue_transpose(0)
            issue_qmult(1)
            for s in range(NQTR + 5):
                if s % 4 == 0 and s < NQTR:
                    front(s // 4)
                if s < NQTR:
                    st_l1(s)
                if 1 <= s < NQTR + 1:
                    st_relu1(s - 1)
                if 2 <= s < NQTR + 2:
                    st_l2(s - 2)
                if 5 <= s < NQTR + 5:
                    st_l3(s - 5)

    return nc


# ======================= host-side preparation ==========================

def _host_pointdata(x):
    """Exact fp32 replication of the reference's per-point index/weight math.

    Returns (cell[int64], c4[N,4] fp32 in corner order 00,10,01,11, idf)."""
    u = np.asarray(x[:, 1], np.float32)
    v = np.asarray(x[:, 2], np.float32)
    xu = u * np.float32(RX)
    yv = v * np.float32(RY)
    x0 = xu.astype(np.int32)
    x0 = np.where(x0 == RX, 0, x0)
    y0 = yv.astype(np.int32)
    wx = xu - x0.astype(np.float32)
    wy = yv - y0.astype(np.float32)
    cell = np.minimum(y0.astype(np.int64) * RX + x0, RX * RY - 1)
    one = np.float32(1.0)
    c00 = (one - wx) * (one - wy)
    c10 = wx * (one - wy)
    c01 = (one - wx) * wy
    c11 = wx * wy
    c4 = np.stack([c00, c10, c01, c11], axis=1)
    return cell, c4, np.asarray(x[:, 0], np.float32)


def _patch_table16(emb):
    """[RX*RY + 8, 16] fp16: per-cell patch in [f][j] order (j: 00,10,01,11),
    with x/y edge clamping baked in; 8 zero pad rows."""
    e = np.asarray(emb, dtype=np.float32).reshape(RY, RX, F)
    xs = np.arange(RX)
    x1 = np.minimum(xs + 1, RX - 1)
    ys = np.arange(RY)
    y1 = np.minimum(ys + 1, RY - 1)
    p = np.empty((RY, RX, F, 4), dtype=np.float32)   # [y, x, f, j]
    p[:, :, :, 0] = e
    p[:, :, :, 1] = e[:, x1, :]
    p[:, :, :, 2] = e[y1, :, :]
    p[:, :, :, 3] = e[y1][:, x1, :]
    pt = np.zeros((RY * RX + 8, 16), dtype=np.float16)
    pt[:RY * RX] = p.reshape(RY * RX, 16).astype(np.float16)
    return pt


def _pairing(cell_sorted):
    """Pair consecutive-cell points. Returns (pA, pB, base) index arrays into
    the SORTED order; pB == -1 for half-dummy pairs."""
    N = len(cell_sorted)
    counts = np.bincount(cell_sorted, minlength=RX * RY)
    starts = np.zeros(RX * RY + 1, np.int64)
    np.cumsum(counts, out=starts[1:])
    wi = np.arange(N) - starts[cell_sorted]          # within-cell rank
    cnt = counts[cell_sorted]
    inA = (wi % 2 == 0) & (wi + 1 < cnt)             # within-cell pair firsts
    pA_in = np.nonzero(inA)[0]
    pB_in = pA_in + 1
    # leftover points (one per odd-count cell), ordered by cell
    lmask = (wi == cnt - 1) & (cnt % 2 == 1)
    lpos = np.nonzero(lmask)[0]
    lcell = cell_sorted[lpos]
    # pair leftovers within runs of consecutive cells
    if len(lpos):
        newrun = np.r_[True, np.diff(lcell) != 1]
        runid = np.cumsum(newrun) - 1
        rstart = np.nonzero(newrun)[0]
        within = np.arange(len(lpos)) - rstart[runid]
        runlen = np.bincount(runid)
        isA = (within % 2 == 0) & (within + 1 < runlen[runid])
        pA_lo = lpos[isA]
        pB_lo = lpos[np.nonzero(isA)[0] + 1]
        solo = (within == runlen[runid] - 1) & (runlen[runid] % 2 == 1)
        pA_solo = lpos[solo]
    else:
        pA_lo = pB_lo = pA_solo = np.zeros(0, np.int64)
    pA = np.concatenate([pA_in, pA_lo, pA_solo])
    pB = np.concatenate([pB_in, pB_lo, np.full(len(pA_solo), -1, np.int64)])
    base = cell_sorted[pA]
    o = np.argsort(base, kind="stable")
    return pA[o], pB[o], base[o]


def _host_prep_weights(w1, b1, w2, b2, w3, b3):
    w1 = np.asarray(w1, np.float32)
    # w1x rows s = h*16 + f*4 + j -> w1[1+f]
    w1x = np.zeros((SLOT, HID), np.float32)
    for h in range(2):
        for f in range(4):
            for j in range(4):
                w1x[h * 16 + f * 4 + j] = w1[1 + f]
    lhsT1 = np.zeros((67, 128), np.float16)
    lhsT1[0:32, 0:64] = w1x          # even point slots
    lhsT1[32:64, 64:128] = w1x       # odd point slots
    lhsT1[64, 0:64] = w1[0]          # even idf
    lhsT1[65, 64:128] = w1[0]        # odd idf
    lhsT1[66, 0:64] = np.asarray(b1, np.float32)   # ones row -> +b1
    lhsT1[66, 64:128] = np.asarray(b1, np.float32)
    lhsT2 = np.zeros((128, 128), np.float16)
    lhsT2[0:64, 0:64] = w2
    lhsT2[64:128, 64:128] = w2
    lhsT3 = np.zeros((128, 96), np.float16)
    for e in range(4):
        lhsT3[0:64, 24 * e + 6 * e:24 * e + 6 * e + 3] = w3
        lhsT3[64:128, 24 * e + 6 * e + 3:24 * e + 6 * e + 6] = w3
    b2rep = np.concatenate([b2, b2]).astype(np.float32).reshape(128, 1)
    b3rep = np.zeros((24, 1), np.float32)
    for e in range(4):
        b3rep[6 * e:6 * e + 3, 0] = b3
        b3rep[6 * e + 3:6 * e + 6, 0] = b3
    return lhsT1, lhsT2, lhsT3, b2rep, b3rep


def _colmaps():
    """Column maps for the v3 psum/output layout.

    Returns jc_e: [4, 1024] int64 — for output column v in [0,1024) and
    row-block e in [0,4): the within-chunk PAIR index jc whose points occupy
    y3c rows 6e..6e+3 (A) and 6e+3..6e+6 (B) of column v."""
    v = np.arange(1024)
    t = v // 256          # h2 tile (= psum quarter) index
    uu = v % 256          # octet index within the tile
    jc_e = np.empty((4, 1024), np.int64)
    for e in range(4):
        Cc = 1024 * t + 4 * uu + e          # chunk psum column
        qq = Cc // 1024
        rr = Cc % 1024
        half = rr // 512
        cc = (rr % 512) // 128
        p = rr % 128
        g = 8 * qq + 4 * half + cc
        jc_e[e] = g * 128 + p
    return jc_e


def _prep_in_maps(x, emb, w1, b1, w2, b2, w3, b3):
    x = np.asarray(x, np.float32)
    cell, c4, idf = _host_pointdata(x)
    order = np.argsort(cell, kind="stable")
    cs = cell[order]
    pA_s, pB_s, base = _pairing(cs)          # indices into sorted order
    npair_real = len(pA_s)

    n_chunks = -(-npair_real // (N_CORES * CHPAIRS))
    npairs = n_chunks * N_CORES * CHPAIRS
    PAIRS_CORE = n_chunks * CHPAIRS

    # orig-index pair arrays, padded with full dummies
    pA = np.full(npairs, -1, np.int64)
    pB = np.full(npairs, -1, np.int64)
    bases = np.full(npairs, int(base[-1]) if npair_real else 0, np.int64)
    pA[:npair_real] = order[pA_s]
    pB[:npair_real] = np.where(pB_s >= 0, order[np.maximum(pB_s, 0)], -1)
    bases[:npair_real] = base

    # per-point halves: h for B points (0 or 1); A is always h=0
    validB = pB >= 0
    hB = np.zeros(npairs, np.int64)
    hB[validB] = cell[pB[validB]] - bases[validB]
    assert hB.min() >= 0 and hB.max() <= 1

    # c8 per pair half [npairs, 2, 8] fp16
    c8 = np.zeros((npairs, 2, 8), np.float16)
    vA = pA >= 0
    c8[vA, 0, 0:4] = c4[pA[vA]].astype(np.float16)
    c8[validB, 1, 0:4] = np.where((hB[validB] == 0)[:, None],
                                  c4[pB[validB]], 0).astype(np.float16)
    c8[validB, 1, 4:8] = np.where((hB[validB] == 1)[:, None],
                                  c4[pB[validB]], 0).astype(np.float16)
    idfA = np.where(vA, idf[np.maximum(pA, 0)], 0).astype(np.float16)
    idfB = np.where(validB, idf[np.maximum(pB, 0)], 0).astype(np.float16)

    pt16 = _patch_table16(emb)
    # densely prepacked per-pair patch blocks: [npairs, 32] fp16
    pp32 = np.empty((npairs, 2 * 16), np.float16)
    pp32[:, 0:16] = pt16[bases]
    pp32[:, 16:32] = pt16[bases + 1]

    lhsT1, lhsT2, lhsT3, b2rep, b3rep = _host_prep_weights(
        w1, b1, w2, b2, w3, b3)
    jc_e = _colmaps()

    in_maps = []
    out_pairs = []                            # (origA[4,ncol], origB) per core
    for k in range(N_CORES):
        s = k * PAIRS_CORE
        # ptw: [n_chunks*P, GCH*SLOT]: row (c, p), block g = pair jc=g*128+p
        kpp = pp32[s:s + PAIRS_CORE].reshape(n_chunks, GCH, P, SLOT)
        ptw = np.ascontiguousarray(
            kpp.transpose(0, 2, 1, 3).reshape(n_chunks * P, GCH * SLOT))

        # co8: [P, n_chunks*64, 8] : pair jc=(g*128+p) -> co8[p, chunk*64+2g+d]
        kc8 = c8[s:s + PAIRS_CORE].reshape(n_chunks, GCH, P, 2, 8)
        co8 = np.ascontiguousarray(
            kc8.transpose(2, 0, 1, 3, 4).reshape(P, n_chunks * TCH, 8))

        # idfc: [P, n_chunks*GCH*2]: idfc[p, (c, g, d)] = idf of point d of
        # pair jc = c*4096 + g*128 + p
        kiA = idfA[s:s + PAIRS_CORE].reshape(n_chunks, GCH, P)
        kiB = idfB[s:s + PAIRS_CORE].reshape(n_chunks, GCH, P)
        kid = np.stack([kiA, kiB], axis=3)        # [c, g, p, d] -> wait
        # kid[c, g, p, d]; want idfc[p, c, g, d]
        idfc = np.ascontiguousarray(
            kid.transpose(2, 0, 1, 3).reshape(P, n_chunks * GCH * 2))

        # output column -> orig point indices: [4, n_chunks*1024] each
        kpA = pA[s:s + PAIRS_CORE].reshape(n_chunks, CHPAIRS)
        kpB = pB[s:s + PAIRS_CORE].reshape(n_chunks, CHPAIRS)
        oA = np.empty((4, n_chunks * 1024), np.int64)
        oB = np.empty((4, n_chunks * 1024), np.int64)
        for e in range(4):
            oA[e] = kpA[:, jc_e[e]].reshape(-1)
            oB[e] = kpB[:, jc_e[e]].reshape(-1)
        out_pairs.append((oA, oB))

        in_maps.append({
            "ptw": ptw,
            "co8": co8,
            "idfc": idfc,
            "lhsT1": lhsT1,
            "lhsT2": lhsT2,
            "lhsT3": lhsT3,
            "b2rep": b2rep,
            "b3rep": b3rep,
        })
    return in_maps, out_pairs, n_chunks


_CACHE = {}


def kernel(x, emb, w1, b1, w2, b2, w3, b3):
    from concourse.bass_utils import run_bass_kernel_spmd

    x = np.asarray(x, np.float32)
    N = x.shape[0]

    in_maps, out_pairs, n_chunks = _prep_in_maps(x, emb, w1, b1, w2, b2, w3, b3)

    key = (n_chunks,)
    if key not in _CACHE:
        nc_new = _build_bass(n_chunks)
        nc_new.compile()
        _CACHE[key] = nc_new
    nc = _CACHE[key]

    trace = os.environ.get("KERNEL_TRACE", "0") == "1"
    res = run_bass_kernel_spmd(
        nc, in_maps, core_ids=list(range(N_CORES)), trace=trace
    )
    if trace and res.exec_time_ns is not None:
        print(f"HW exec time: {res.exec_time_ns} ns")

    y = np.empty((N, 3), np.float32)
    for k in range(N_CORES):
        yT = np.asarray(res.results[k]["yT"], np.float32)
        oA, oB = out_pairs[k]
        for e in range(4):
            mA = oA[e] >= 0
            mB = oB[e] >= 0
            y[oA[e][mA], :] = yT[6 * e:6 * e + 3, mA].T
            y[oB[e][mB], :] = yT[6 * e + 3:6 * e + 6, mB].T
    return y


# revision 21
# speedup vs baseline: 1.4982x; 1.3839x over previous
# kernel.py — Trainium2 Bass kernel for nn_DenseGridNet (bilinear grid sample + MLP)
#
# v3.1 strategy (data-parallel over 8 NeuronCores, sorted + PAIRED points):
#  * host: computes cell ids + bilinear coefs (exact fp32 replication of the
#    reference), sorts points by cell, PAIRS adjacent points whose cells are
#    <=1 apart (~3% dummy padding), and uploads:
#      - ptw: DENSELY PREPACKED per-pair 64B patch blocks (the pair's two
#        16-value cell patches), laid out [chunk][partition][pair-in-row].
#        This replaces v2's windowed dma_gather (256B descriptors, 2x small-
#        transfer penalty, SWDGE descriptor generation on GpSimd) with a
#        plain contiguous 2KB-per-partition DMA: ~8x less DMA-engine time.
#      - fp16 c8 coefs (4 bilinear coefs placed in the pair-half the point
#        occupies, zeros in the other half),
#      - fp16 idf values in a partition-major layout.
#  * device per 4096-pair chunk:
#      - one contiguous DMA loads the 64B patch blocks -> pb [128,32,32];
#      - GpSimd fp16 multiply q = c8 (x) patch, written STRIDED into a
#        128-slot-per-pair q tile: slots 0:64 = q products, 64:66 = the
#        pair's idf values (tiny DVE copy), 66 = constant 1.0 (memset once),
#        67:128 = zeros (memset once);
#      - one xbar DMA transpose of the q tile: because the pair unit is 128
#        slots, pair-slot s lands exactly at partition s, so the idf/ones
#        rows ride THROUGH the transpose and layer1 needs a SINGLE K=67
#        matmul per 512 columns (w1x slots + idf*w1[0] + b1 fold): half of
#        v2's layer-1 PE passes, and no extra K=2 idf matmul;
#      - layer2 block-diagonal (2 points/col, K=128);
#      - layer3 packs 8 points/col: 4 accumulating stride-4 matmuls into a
#        [24,256] psum tile, so sigmoid free-size is 4x smaller than v2;
#      - relu work split across DVE/ACT, qmult on GpSimd (it cannot read
#        PSUM so it gets no relu work), sigmoid on ACT, fp16 output.
import os
import numpy as np

RX = 1024
RY = 1024
F = 4
HID = 64
N_CORES = 8
P = 128          # partitions
SLOT = 32        # fp16 q slots per point ([h][f][j] of the 2-cell pair)
PU = 128         # padded pair unit in the q tile (64 q + idfA + idfB + 1 + pad)
CHPAIRS = 4096   # pairs per chunk
CHPTS = 8192     # points per chunk (2 per pair, incl. dummies)
GCH = CHPAIRS // P            # 32 pair rows per partition per chunk
TCH = CHPTS // P              # 64 point cols per chunk


def _build_bass(n_chunks, front_swap=False):
    """Bass program for one core processing n_chunks*CHPTS (padded) points."""
    import concourse.bass as bass
    import concourse.tile as tile
    from concourse import bacc
    import concourse.mybir as mybir

    dt = mybir.dt

    nc = bacc.Bacc(None, target_bir_lowering=False)

    f32 = dt.float32
    f16 = dt.float16

    # ---- DRAM I/O -------------------------------------------------------
    ptw_d = nc.dram_tensor("ptw", [n_chunks * P, GCH * SLOT], f16,
                           kind="ExternalInput")
    co8_d = nc.dram_tensor("co8", [P, n_chunks * TCH, 8], f16,
                           kind="ExternalInput")
    idfc_d = nc.dram_tensor("idfc", [P, n_chunks * GCH * 2], f16,
                            kind="ExternalInput")
    l1_d = nc.dram_tensor("lhsT1", [67, 128], f16, kind="ExternalInput")
    l2_d = nc.dram_tensor("lhsT2", [128, 128], f16, kind="ExternalInput")
    l3_d = nc.dram_tensor("lhsT3", [128, 96], f16, kind="ExternalInput")
    b2_d = nc.dram_tensor("b2rep", [128, 1], f32, kind="ExternalInput")
    b3_d = nc.dram_tensor("b3rep", [24, 1], f32, kind="ExternalInput")
    out_d = nc.dram_tensor("yT", [24, n_chunks * 1024], f16,
                           kind="ExternalOutput")

    with tile.TileContext(nc) as tc:
        with (
            tc.tile_pool(name="persist", bufs=1) as pp,
            tc.tile_pool(name="psum_l1", bufs=2, space="PSUM") as ps1,
            tc.tile_pool(name="psum_l2", bufs=2, space="PSUM") as ps2,
            tc.tile_pool(name="psum_l3", bufs=2, space="PSUM") as ps3,
            tc.tile_pool(name="mlp", bufs=3) as mp,
            tc.tile_pool(name="mlp2", bufs=4) as mp2,
            tc.tile_pool(name="outp", bufs=3) as op_,
        ):
            l1w = pp.tile([67, 128], f16, tag="l1w")
            l2w = pp.tile([128, 128], f16, tag="l2w")
            l3w = pp.tile([128, 96], f16, tag="l3w")
            b2r = pp.tile([128, 1], f32, tag="b2r")
            b3r = pp.tile([24, 1], f32, tag="b3r")

            NPB = 8
            NQ = 4
            NTB = 4
            pb = [pp.tile([P, GCH, SLOT], f16, name=f"pb{i}", tag=f"pb{i}")
                  for i in range(NPB)]
            qpm = [pp.tile([P, GCH, PU], f16, name=f"qpm{i}", tag=f"qpm{i}")
                   for i in range(NQ)]
            tb = [pp.tile([P, GCH, P], f16, name=f"tb{i}", tag=f"tb{i}")
                  for i in range(NTB)]
            co8s = pp.tile([P, n_chunks * TCH, 8], f16, tag="co8s")
            idfc = pp.tile([P, n_chunks * GCH * 2], f16, tag="idfc")

            nc.sync.dma_start(l1w[:], l1_d[:])
            nc.sync.dma_start(l2w[:], l2_d[:])
            nc.sync.dma_start(l3w[:], l3_d[:])
            nc.sync.dma_start(b2r[:], b2_d[:])
            nc.sync.dma_start(b3r[:], b3_d[:])
            nc.sync.dma_start(co8s[:], co8_d[:])
            nc.sync.dma_start(idfc[:], idfc_d[:])
            for i in range(NQ):
                # pair-unit slots 66 (=1.0 for the b1 fold) and 67:128
                # (zeros) are constant across chunks: written once.
                nc.vector.memset(qpm[i][:, :, 66:67], 1.0)
                nc.vector.memset(qpm[i][:, :, 67:PU], 0.0)

            AL = mybir.AluOpType
            ACTF = mybir.ActivationFunctionType

            def issue_pb(chi):
                """patch-pair block load for chunk chi. All DMAs are issued
                from SP: a DMA's semaphore waits occupy the issuing engine's
                in-order sequencer, so putting DMAs on ACT/DVE would block
                their compute queues behind DMA dependencies."""
                nc.sync.dma_start(
                    pb[chi % NPB][:],
                    ptw_d[chi * P:(chi + 1) * P, :]
                    .rearrange("p (g s) -> p g s", s=SLOT))

            def issue_qmult(chi):
                """q = c8 (x) patch for chunk chi, plus the idf slot copy."""
                pbb = pb[chi % NPB]
                qb = qpm[chi % NQ]
                c0 = chi * TCH
                in1 = (co8s[:, c0:c0 + TCH, :]
                       .rearrange("p (g d) (h j) -> p g d h j", d=2, h=2)
                       .unsqueeze(4)
                       .to_broadcast([P, GCH, 2, 2, 4, 4]))
                in2 = (pbb[:]
                       .rearrange("p g (h f j) -> p g h f j", h=2, f=4)
                       .unsqueeze(2)
                       .to_broadcast([P, GCH, 2, 2, 4, 4]))
                outq = (qb[:, :, 0:64]
                        .rearrange("p g (d h f j) -> p g d h f j",
                                   d=2, h=2, f=4))
                # GpSimd: the only engine with no psum-side relu work
                # (gpsimd cannot read PSUM), so q-mult goes here.
                nc.gpsimd.tensor_tensor(outq, in1, in2, AL.mult)
                # pair idf values -> slots 64:66 (ride through the transpose)
                nc.vector.tensor_copy(
                    qb[:, :, 64:66],
                    idfc[:, chi * GCH * 2:(chi + 1) * GCH * 2]
                    .rearrange("p (g d) -> p g d", d=2))

            def issue_transpose(chi):
                nc.sync.dma_start_transpose(
                    tb[chi % NTB][:],
                    qpm[chi % NQ][:].rearrange("p g s -> p (g s)"))

            # two-deep front pipeline: the q chain (qmult ~4.2us + transpose
            # ~3.6us) exceeds one chunk period, so qmult runs two chunks
            # ahead of the MLP and the transpose one chunk ahead.
            # ---- flat software pipeline over global quarter-steps --------
            # Engines execute their queues IN ORDER, so chunk-sequential
            # emission makes every chunk's slow tail (L3->sigmoid) block the
            # next chunk's head. Instead, one step s emits: L1 of quarter s,
            # relu1 of s-1, L2+relu2 of s-2, L3(+sigmoid) of s-4 — each
            # engine's queue then interleaves 3-4 chunks and every
            # dependency is satisfied about a full step before its consumer.
            H = 512
            NQTR = 4 * n_chunks
            l1pp = {}
            h1t = {}
            h2t = {}
            l3pt = {}
            y3t = {}

            def front(k):
                if k + 2 < n_chunks:
                    issue_qmult(k + 2)
                if k + 1 < n_chunks:
                    issue_transpose(k + 1)
                if k + 5 < n_chunks:
                    issue_pb(k + 5)

            def st_l1(Q):
                k, qq = Q // 4, Q % 4
                tbb = tb[k % NTB]
                l1p = ps1.tile([P, 1024], f32, name="l1p", tag="l1p")
                l1pp[Q] = l1p
                for half in range(2):
                    m = 2 * qq + half
                    # single K=67 pass: q slots + idf rows + ones(b1)
                    nc.tensor.matmul(l1p[:, half * H:half * H + H],
                                     l1w[:],
                                     tbb[0:67, 4 * m:4 * m + 4, :],
                                     start=True, stop=True)

            def st_relu1(Q):
                l1p = l1pp.pop(Q)
                h1 = mp.tile([P, 1024], f16, tag="h1")
                # relu1: b1 already folded into the matmul ones-row
                if Q % 4 == 0:
                    nc.vector.tensor_scalar(h1[:], l1p[:], 0.0, None, AL.max)
                else:
                    nc.scalar.activation(h1[:], l1p[:], ACTF.Relu)
                h1t[Q] = h1

            def st_l2(Q):
                h1 = h1t.pop(Q)
                h2 = mp2.tile([P, 1024], f16, name="h2", tag="h2")
                h2t[Q] = h2
                for hh in range(2):
                    l2p = ps2.tile([P, H], f32, tag="l2p")
                    nc.tensor.matmul(l2p[:], l2w[:],
                                     h1[:, hh * H:hh * H + H],
                                     start=True, stop=True)
                    if Q % 2 == 1 and hh == 1:
                        nc.scalar.activation(
                            h2[:, hh * H:hh * H + H], l2p[:],
                            ACTF.Relu, bias=b2r[:])
                    else:
                        nc.vector.tensor_scalar(
                            h2[:, hh * H:hh * H + H], l2p[:],
                            b2r[:], 0.0, AL.add, AL.max)

            def st_l3(Q):
                k, qq = Q // 4, Q % 4
                r, rh = qq // 2, qq % 2
                h2 = h2t.pop(Q)
                if rh == 0:
                    l3pt[Q // 2] = ps3.tile([24, 512], f32, name="l3p",
                                            tag="l3p")
                if qq == 0:
                    y3t[k] = op_.tile([24, 1024], f16, name="y3c", tag="y3c")
                l3pp = l3pt[Q // 2]
                h2v = h2[:].rearrange("p (u e) -> p e u", e=4)
                # one 8-matmul accumulation group per [24,512] bank:
                # quarters 2r (cols 0:256) and 2r+1 (cols 256:512)
                for e in range(4):
                    nc.tensor.matmul(
                        l3pp[:, rh * 256:rh * 256 + 256],
                        l3w[:, 24 * e:24 * e + 24],
                        h2v[:, e, :],
                        start=(rh == 0 and e == 0),
                        stop=(rh == 1 and e == 3),
                        skip_group_check=True)
                if rh == 1:
                    l3pp = l3pt.pop(Q // 2)
                    nc.scalar.activation(
                        y3t[k][:, 512 * r:512 * r + 512],
                        l3pp[:], ACTF.Sigmoid, bias=b3r[:])
                if qq == 3:
                    y3c = y3t.pop(k)
                    nc.sync.dma_start(
                        out_d[:, k * 1024:(k + 1) * 1024], y3c[:])

            for i in range(5):
                if i < n_chunks:
                    issue_pb(i)
            issue_qmult(0)
            issue_transpose(0)
            issue_qmult(1)
            for s in range(NQTR + 5):
                if s % 4 == 0 and s < NQTR:
                    front(s // 4)
                if s < NQTR:
                    st_l1(s)
                if 1 <= s < NQTR + 1:
                    st_relu1(s - 1)
                if 2 <= s < NQTR + 2:
                    st_l2(s - 2)
                if 5 <= s < NQTR + 5:
                    st_l3(s - 5)

    return nc


# ======================= host-side preparation ==========================

def _host_pointdata(x):
    """Exact fp32 replication of the reference's per-point index/weight math.

    Returns (cell[int64], c4[N,4] fp32 in corner order 00,10,01,11, idf)."""
    u = np.asarray(x[:, 1], np.float32)
    v = np.asarray(x[:, 2], np.float32)
    xu = u * np.float32(RX)
    yv = v * np.float32(RY)
    x0 = xu.astype(np.int32)
    x0 = np.where(x0 == RX, 0, x0)
    y0 = yv.astype(np.int32)
    wx = xu - x0.astype(np.float32)
    wy = yv - y0.astype(np.float32)
    cell = np.minimum(y0.astype(np.int64) * RX + x0, RX * RY - 1)
    one = np.float32(1.0)
    c00 = (one - wx) * (one - wy)
    c10 = wx * (one - wy)
    c01 = (one - wx) * wy
    c11 = wx * wy
    c4 = np.stack([c00, c10, c01, c11], axis=1)
    return cell, c4, np.asarray(x[:, 0], np.float32)


def _patch_table16(emb):
    """[RX*RY + 8, 16] fp16: per-cell patch in [f][j] order (j: 00,10,01,11),
    with x/y edge clamping baked in; 8 zero pad rows."""
    e = np.asarray(emb, dtype=np.float32).reshape(RY, RX, F)
    xs = np.arange(RX)
    x1 = np.minimum(xs + 1, RX - 1)
    ys = np.arange(RY)
    y1 = np.minimum(ys + 1, RY - 1)
    p = np.empty((RY, RX, F, 4), dtype=np.float32)   # [y, x, f, j]
    p[:, :, :, 0] = e
    p[:, :, :, 1] = e[:, x1, :]
    p[:, :, :, 2] = e[y1, :, :]
    p[:, :, :, 3] = e[y1][:, x1, :]
    pt = np.zeros((RY * RX + 8, 16), dtype=np.float16)
    pt[:RY * RX] = p.reshape(RY * RX, 16).astype(np.float16)
    return pt


def _pairing(cell_sorted):
    """Pair consecutive-cell points. Returns (pA, pB, base) index arrays into
    the SORTED order; pB == -1 for half-dummy pairs."""
    N = len(cell_sorted)
    counts = np.bincount(cell_sorted, minlength=RX * RY)
    starts = np.zeros(RX * RY + 1, np.int64)
    np.cumsum(counts, out=starts[1:])
    wi = np.arange(N) - starts[cell_sorted]          # within-cell rank
    cnt = counts[cell_sorted]
    inA = (wi % 2 == 0) & (wi + 1 < cnt)             # within-cell pair firsts
    pA_in = np.nonzero(inA)[0]
    pB_in = pA_in + 1
    # leftover points (one per odd-count cell), ordered by cell
    lmask = (wi == cnt - 1) & (cnt % 2 == 1)
    lpos = np.nonzero(lmask)[0]
    lcell = cell_sorted[lpos]
    # pair leftovers within runs of consecutive cells
    if len(lpos):
        newrun = np.r_[True, np.diff(lcell) != 1]
        runid = np.cumsum(newrun) - 1
        rstart = np.nonzero(newrun)[0]
        within = np.arange(len(lpos)) - rstart[runid]
        runlen = np.bincount(runid)
        isA = (within % 2 == 0) & (within + 1 < runlen[runid])
        pA_lo = lpos[isA]
        pB_lo = lpos[np.nonzero(isA)[0] + 1]
        solo = (within == runlen[runid] - 1) & (runlen[runid] % 2 == 1)
        pA_solo = lpos[solo]
    else:
        pA_lo = pB_lo = pA_solo = np.zeros(0, np.int64)
    pA = np.concatenate([pA_in, pA_lo, pA_solo])
    pB = np.concatenate([pB_in, pB_lo, np.full(len(pA_solo), -1, np.int64)])
    base = cell_sorted[pA]
    o = np.argsort(base, kind="stable")
    return pA[o], pB[o], base[o]


def _host_prep_weights(w1, b1, w2, b2, w3, b3):
    w1 = np.asarray(w1, np.float32)
    # w1x rows s = h*16 + f*4 + j -> w1[1+f]
    w1x = np.zeros((SLOT, HID), np.float32)
    for h in range(2):
        for f in range(4):
            for j in range(4):
                w1x[h * 16 + f * 4 + j] = w1[1 + f]
    lhsT1 = np.zeros((67, 128), np.float16)
    lhsT1[0:32, 0:64] = w1x          # even point slots
    lhsT1[32:64, 64:128] = w1x       # odd point slots
    lhsT1[64, 0:64] = w1[0]          # even idf
    lhsT1[65, 64:128] = w1[0]        # odd idf
    lhsT1[66, 0:64] = np.asarray(b1, np.float32)   # ones row -> +b1
    lhsT1[66, 64:128] = np.asarray(b1, np.float32)
    lhsT2 = np.zeros((128, 128), np.float16)
    lhsT2[0:64, 0:64] = w2
    lhsT2[64:128, 64:128] = w2
    lhsT3 = np.zeros((128, 96), np.float16)
    for e in range(4):
        lhsT3[0:64, 24 * e + 6 * e:24 * e + 6 * e + 3] = w3
        lhsT3[64:128, 24 * e + 6 * e + 3:24 * e + 6 * e + 6] = w3
    b2rep = np.concatenate([b2, b2]).astype(np.float32).reshape(128, 1)
    b3rep = np.zeros((24, 1), np.float32)
    for e in range(4):
        b3rep[6 * e:6 * e + 3, 0] = b3
        b3rep[6 * e + 3:6 * e + 6, 0] = b3
    return lhsT1, lhsT2, lhsT3, b2rep, b3rep


def _colmaps():
    """Column maps for the v3 psum/output layout.

    Returns jc_e: [4, 1024] int64 — for output column v in [0,1024) and
    row-block e in [0,4): the within-chunk PAIR index jc whose points occupy
    y3c rows 6e..6e+3 (A) and 6e+3..6e+6 (B) of column v."""
    v = np.arange(1024)
    t = v // 256          # h2 tile (= psum quarter) index
    uu = v % 256          # octet index within the tile
    jc_e = np.empty((4, 1024), np.int64)
    for e in range(4):
        Cc = 1024 * t + 4 * uu + e          # chunk psum column
        qq = Cc // 1024
        rr = Cc % 1024
        half = rr // 512
        cc = (rr % 512) // 128
        p = rr % 128
        g = 8 * qq + 4 * half + cc
        jc_e[e] = g * 128 + p
    return jc_e


def _prep_in_maps(x, emb, w1, b1, w2, b2, w3, b3):
    x = np.asarray(x, np.float32)
    cell, c4, idf = _host_pointdata(x)
    order = np.argsort(cell, kind="stable")
    cs = cell[order]
    pA_s, pB_s, base = _pairing(cs)          # indices into sorted order
    npair_real = len(pA_s)

    n_chunks = -(-npair_real // (N_CORES * CHPAIRS))
    npairs = n_chunks * N_CORES * CHPAIRS
    PAIRS_CORE = n_chunks * CHPAIRS

    # orig-index pair arrays, padded with full dummies
    pA = np.full(npairs, -1, np.int64)
    pB = np.full(npairs, -1, np.int64)
    bases = np.full(npairs, int(base[-1]) if npair_real else 0, np.int64)
    pA[:npair_real] = order[pA_s]
    pB[:npair_real] = np.where(pB_s >= 0, order[np.maximum(pB_s, 0)], -1)
    bases[:npair_real] = base

    # per-point halves: h for B points (0 or 1); A is always h=0
    validB = pB >= 0
    hB = np.zeros(npairs, np.int64)
    hB[validB] = cell[pB[validB]] - bases[validB]
    assert hB.min() >= 0 and hB.max() <= 1

    # c8 per pair half [npairs, 2, 8] fp16
    c8 = np.zeros((npairs, 2, 8), np.float16)
    vA = pA >= 0
    c8[vA, 0, 0:4] = c4[pA[vA]].astype(np.float16)
    c8[validB, 1, 0:4] = np.where((hB[validB] == 0)[:, None],
                                  c4[pB[validB]], 0).astype(np.float16)
    c8[validB, 1, 4:8] = np.where((hB[validB] == 1)[:, None],
                                  c4[pB[validB]], 0).astype(np.float16)
    idfA = np.where(vA, idf[np.maximum(pA, 0)], 0).astype(np.float16)
    idfB = np.where(validB, idf[np.maximum(pB, 0)], 0).astype(np.float16)

    pt16 = _patch_table16(emb)
    # densely prepacked per-pair patch blocks: [npairs, 32] fp16
    pp32 = np.empty((npairs, 2 * 16), np.float16)
    pp32[:, 0:16] = pt16[bases]
    pp32[:, 16:32] = pt16[bases + 1]

    lhsT1, lhsT2, lhsT3, b2rep, b3rep = _host_prep_weights(
        w1, b1, w2, b2, w3, b3)
    jc_e = _colmaps()

    in_maps = []
    out_pairs = []                            # (origA[4,ncol], origB) per core
    for k in range(N_CORES):
        s = k * PAIRS_CORE
        # ptw: [n_chunks*P, GCH*SLOT]: row (c, p), block g = pair jc=g*128+p
        kpp = pp32[s:s + PAIRS_CORE].reshape(n_chunks, GCH, P, SLOT)
        ptw = np.ascontiguousarray(
            kpp.transpose(0, 2, 1, 3).reshape(n_chunks * P, GCH * SLOT))

        # co8: [P, n_chunks*64, 8] : pair jc=(g*128+p) -> co8[p, chunk*64+2g+d]
        kc8 = c8[s:s + PAIRS_CORE].reshape(n_chunks, GCH, P, 2, 8)
        co8 = np.ascontiguousarray(
            kc8.transpose(2, 0, 1, 3, 4).reshape(P, n_chunks * TCH, 8))

        # idfc: [P, n_chunks*GCH*2]: idfc[p, (c, g, d)] = idf of point d of
        # pair jc = c*4096 + g*128 + p
        kiA = idfA[s:s + PAIRS_CORE].reshape(n_chunks, GCH, P)
        kiB = idfB[s:s + PAIRS_CORE].reshape(n_chunks, GCH, P)
        kid = np.stack([kiA, kiB], axis=3)        # [c, g, p, d] -> wait
        # kid[c, g, p, d]; want idfc[p, c, g, d]
        idfc = np.ascontiguousarray(
            kid.transpose(2, 0, 1, 3).reshape(P, n_chunks * GCH * 2))

        # output column -> orig point indices: [4, n_chunks*1024] each
        kpA = pA[s:s + PAIRS_CORE].reshape(n_chunks, CHPAIRS)
        kpB = pB[s:s + PAIRS_CORE].reshape(n_chunks, CHPAIRS)
        oA = np.empty((4, n_chunks * 1024), np.int64)
        oB = np.empty((4, n_chunks * 1024), np.int64)
        for e in range(4):
            oA[e] = kpA[:, jc_e[e]].reshape(-1)
            oB[e] = kpB[:, jc_e[e]].reshape(-1)
        out_pairs.append((oA, oB))

        in_maps.append({
            "ptw": ptw,
            "co8": co8,
            "idfc": idfc,
            "lhsT1": lhsT1,
            "lhsT2": lhsT2,
            "lhsT3": lhsT3,
            "b2rep": b2rep,
            "b3rep": b3rep,
        })
    return in_maps, out_pairs, n_chunks


_CACHE = {}


def kernel(x, emb, w1, b1, w2, b2, w3, b3):
    from concourse.bass_utils import run_bass_kernel_spmd

    x = np.asarray(x, np.float32)
    N = x.shape[0]

    in_maps, out_pairs, n_chunks = _prep_in_maps(x, emb, w1, b1, w2, b2, w3, b3)

    key = (n_chunks,)
    if key not in _CACHE:
        nc_new = _build_bass(n_chunks)
        nc_new.compile()
        _CACHE[key] = nc_new
    nc = _CACHE[key]

    trace = os.environ.get("KERNEL_TRACE", "0") == "1"
    res = run_bass_kernel_spmd(
        nc, in_maps, core_ids=list(range(N_CORES)), trace=trace
    )
    if trace and res.exec_time_ns is not None:
        print(f"HW exec time: {res.exec_time_ns} ns")

    y = np.empty((N, 3), np.float32)
    for k in range(N_CORES):
        yT = np.asarray(res.results[k]["yT"], np.float32)
        oA, oB = out_pairs[k]
        for e in range(4):
            mA = oA[e] >= 0
            mB = oB[e] >= 0
            y[oA[e][mA], :] = yT[6 * e:6 * e + 3, mA].T
            y[oB[e][mB], :] = yT[6 * e + 3:6 * e + 6, mB].T
    return y
